# revision 22
# baseline (speedup 1.0000x reference)
"""Binary tree-LSTM (BinaryTokenTreeModel) Trainium2 kernel, v5.

Problem: complete binary tree, depth 15 (N=32767 nodes), tree-LSTM with
state size 2H=512, gates 4*2H=2048, vocab 32.  Reference processes nodes
leaves-first; node i's input state is the concat of the first H=256 dims
of its two children's states.

Strategy (8 NeuronCores):
  * Data-parallel over 8 subtrees rooted at the 8 level-3 nodes (7..14).
    Each core runs a level-synchronous scan over global levels 13..10 of
    its subtree (1920 nodes per core, 93.75% of the tree with the leaf
    level).  Host: leaf level (32-entry type table, zero arithmetic) and
    the inherently-serial 1023-node top (levels 9..0, exact fp32).
  * VOCAB=32 => x_proj folded into the level matmul as a one-hot
    contraction block (K = 256+256+32 = 544).  Level 13 contracts K=96
    one-hots against a reparameterized table (children are leaves).
  * sigma-everywhere gates: tanh(x) = 2*sigmoid(2x) - 1 with the 2x folded
    into the g-gate weight columns, minimizing ACT instructions (352-cycle
    fixed cost each); real Tanh only for c_new.
  * Gate column layout [i | f | o | g] (each 512 = crit 256 | defer 256) so
    every cell op is a flat contiguous f16 slice.
  * Level 10 computes the critical half first (feeding the host boundary),
    deferred half as a trailing chunk interleaved into the PE idle.
  * PSUM: gates pool 3 x [128,1024]x2banks, transpose scratch pool
    2 x 1 bank -- feeds never block the matmul ring.
  * Feed transposes use permuted identities so A/B-child columns come out
    blocked; all feed copies are contiguous.  f16 everywhere off-PSUM.

Self-contained: hardcodes all shapes; only needs numpy + the concourse
(bass) toolchain that ships with the environment.
"""

import sys

for _p in ("/opt/trn_rl_repo", "/root/.axon_site/_ro/trn_rl_repo"):
    if _p not in sys.path:
        sys.path.append(_p)

import numpy as np

import concourse.bacc as bacc
import concourse.mybir as mybir
import concourse.tile as tile
from concourse.bass_utils import run_bass_kernel_spmd

F32 = mybir.dt.float32
F16 = mybir.dt.float16
AF = mybir.ActivationFunctionType
ALU = mybir.AluOpType

N_CORES = 8
N = 32767
H = 256
H2 = 512
G = 2048  # 4 * H2
V = 32
LEAF0 = (1 << 14) - 1  # 16383: first leaf node id

# Gate column order [i | f | o | g]; orig torch row order is i f g o.
PERM3 = np.concatenate([
    np.arange(0, 512), np.arange(512, 1024),
    np.arange(1536, 2048), np.arange(1024, 1536),
])

DEV_PLAN = [(13, 1024, 0), (12, 512, 1024), (11, 256, 1536), (10, 128, 1792)]
OUT_ROWS = 2048  # 1920 h rows + 128 rows of level-10 c_crit (cols 0:256)
OHS_OFF = {12: 0, 11: 512, 10: 768}
OHS_W = 896
EYP_OFF = {128: 0}
EYP_W = 128

_BUILT = None  # cached (nc, input_names)
LAST_RESULTS = None  # BassKernelResults of the most recent run (for profiling)


def _sigmoid(x):
    return 1.0 / (1.0 + np.exp(-x))


class _Stor:
    def __init__(self, sA0, sA1, sB0, sB1, cin):
        self.sA0, self.sA1, self.sB0, self.sB1, self.cin = sA0, sA1, sB0, sB1, cin


def _build_program(nc):
    din = {}
    for name, shape in [
        ("wk0", [128, G]), ("wk1", [128, G]), ("wk2", [128, G]), ("wk3", [128, G]),
        ("woh", [32, G]), ("w13", [96, G]),
        ("oh3", [96, 1024]), ("ohs", [32, OHS_W]),
        ("eyp", [128, EYP_W]), ("cin13", [1024, 512]),
    ]:
        din[name] = nc.dram_tensor(name, shape, F16, kind="ExternalInput").ap()
    out_d = nc.dram_tensor("out", [OUT_ROWS, 512], F16, kind="ExternalOutput").ap()

    sbh = lambda n, sh: nc.alloc_sbuf_tensor(n, sh, F16).ap()
    wk = [sbh(f"wk{i}_s", [128, G]) for i in range(4)]
    woh_s = sbh("woh_s", [32, G])
    w13_s = sbh("w13_s", [96, G])
    oh3_s = sbh("oh3_s", [96, 1024])
    ohs_s = sbh("ohs_s", [32, OHS_W])
    eyp_s = sbh("eyp_s", [128, EYP_W])
    cin13_s = sbh("cin13_s", [128, 8 * 512])

    stor = {}
    for L, M in [(12, 512), (11, 256), (10, 128)]:
        mk = lambda nm: sbh(f"{nm}_{L}", [128, M])
        stor[L] = _Stor(mk("sA0"), mk("sA1"), mk("sB0"), mk("sB1"),
                        sbh(f"cin_{L}", [128, (M // 128) * 512]))

    with tile.TileContext(nc) as tc:
        import contextlib

        with contextlib.ExitStack() as ctx:
            g_pool = ctx.enter_context(
                tc.tile_pool(name="g", bufs=3, space="PSUM"))
            tr_pool = ctx.enter_context(
                tc.tile_pool(name="tr", bufs=2, space="PSUM"))
            sig_pool = ctx.enter_context(tc.tile_pool(name="sig", bufs=4))
            cell_pool = ctx.enter_context(tc.tile_pool(name="cell", bufs=3))

            # input loads spread over three DGE queues; L13's operands first
            nc.sync.dma_start(eyp_s, din["eyp"])
            nc.sync.dma_start(w13_s[0:48], din["w13"][0:48])
            nc.scalar.dma_start(w13_s[48:96], din["w13"][48:96])
            nc.sync.dma_start(oh3_s[0:48], din["oh3"][0:48])
            nc.scalar.dma_start(oh3_s[48:96], din["oh3"][48:96])
            for k in range(8):
                nc.gpsimd.dma_start(cin13_s[:, k * 512:(k + 1) * 512],
                                    din["cin13"][k * 128:(k + 1) * 128, :])
            nc.sync.dma_start(wk[0], din["wk0"])
            nc.scalar.dma_start(wk[1], din["wk1"])
            nc.sync.dma_start(wk[2], din["wk2"])
            nc.scalar.dma_start(wk[3], din["wk3"])
            nc.gpsimd.dma_start(woh_s, din["woh"])
            nc.gpsimd.dma_start(ohs_s, din["ohs"])

            # junk matmuls: occupy the HAM activity window while input DMAs
            # land so the PE unthrottles to 2.4 GHz before the real work
            wtile = g_pool.tile([128, 1024], F32, tag="g")
            for _ in range(16):
                nc.tensor.matmul(wtile[0:128, 0:128], eyp_s[:, 0:128],
                                 eyp_s[:, 0:128], start=True, stop=True,
                                 skip_group_check=True)

            def emit_fused(gA, gB, lhs, ws, P):
                nk = len(lhs)
                for k in range(nk):
                    st, sp = k == 0, k == nk - 1
                    for gt, wc in ((gA, 0), (gA, 512), (gB, 1024), (gB, 1536)):
                        oc = wc % 1024
                        nc.tensor.matmul(gt[0:P, oc:oc + 512], lhs[k],
                                         ws[k][:, wc:wc + 512],
                                         start=st, stop=sp,
                                         skip_group_check=True)

            def emit_half(g, lhs, ws, dsel, P):
                nk = len(lhs)
                for k in range(nk):
                    st, sp = k == 0, k == nk - 1
                    for j, wc in enumerate((0, 512, 1024, 1536)):
                        w0 = wc + dsel * 256
                        nc.tensor.matmul(g[0:P, j * 256:(j + 1) * 256], lhs[k],
                                         ws[k][:, w0:w0 + 256],
                                         start=st, stop=sp,
                                         skip_group_check=True)

            def unit_fused(L, pk, row_off):
                P = 128
                c0 = pk * 128
                gA = g_pool.tile([128, 1024], F32, tag="g")
                gB = g_pool.tile([128, 1024], F32, tag="g")
                if L == 13:
                    lhs = [oh3_s[:, c0:c0 + P]]
                    ws = [w13_s]
                    cin_ap = cin13_s[0:P, pk * 512:(pk + 1) * 512]
                else:
                    st = stor[L]
                    lhs = [st.sA0[:, c0:c0 + P], st.sA1[:, c0:c0 + P],
                           st.sB0[:, c0:c0 + P], st.sB1[:, c0:c0 + P],
                           ohs_s[:, OHS_OFF[L] + c0:OHS_OFF[L] + c0 + P]]
                    ws = wk + [woh_s]
                    cin_ap = st.cin[0:P, pk * 512:(pk + 1) * 512]
                emit_fused(gA, gB, lhs, ws, P)

                sg = sig_pool.tile([128, 2048], F16, tag="sg")
                nc.scalar.activation(sg[0:P, 0:1024], gA[0:P], AF.Sigmoid)
                nc.scalar.activation(sg[0:P, 1024:2048], gB[0:P], AF.Sigmoid)
                i_ = sg[0:P, 0:512]
                f_ = sg[0:P, 512:1024]
                o_ = sg[0:P, 1024:1536]
                g_ = sg[0:P, 1536:2048]
                q = cell_pool.tile([128, 512], F16, tag="q")
                nc.gpsimd.tensor_mul(q[0:P], f_, cin_ap)
                p = cell_pool.tile([128, 512], F16, tag="p")
                nc.vector.tensor_mul(p[0:P], i_, g_)
                pr = cell_pool.tile([128, 512], F16, tag="pr")
                nc.vector.scalar_tensor_tensor(pr[0:P], p[0:P], 2.0, i_,
                                               ALU.mult, ALU.subtract)
                cn = cell_pool.tile([128, 512], F16, tag="cn", bufs=3)
                nc.vector.tensor_add(cn[0:P], q[0:P], pr[0:P])
                tc_ = cell_pool.tile([128, 512], F16, tag="tc")
                nc.scalar.activation(tc_[0:P], cn[0:P], AF.Tanh)
                hn = cell_pool.tile([128, 512], F16, tag="hn", bufs=3)
                nc.vector.tensor_mul(hn[0:P], o_, tc_[0:P])
                nc.gpsimd.dma_start(out_d[row_off + c0:row_off + c0 + P, :],
                                    hn[0:P])
                return (hn, cn, P)

            def unit_half(lhs, cin_half, P, dsel, out_ap):
                """Critical (dsel=0) or deferred (dsel=1) half of level 10;
                gates [i f o g] (256 each) in one 1024-col tile."""
                g = g_pool.tile([128, 1024], F32, tag="g")
                emit_half(g, lhs, wk + [woh_s], dsel, P)
                sg = sig_pool.tile([128, 1024], F16, tag="sgh")
                nc.scalar.activation(sg[0:P], g[0:P], AF.Sigmoid)
                i_ = sg[0:P, 0:256]
                f_ = sg[0:P, 256:512]
                o_ = sg[0:P, 512:768]
                gg = sg[0:P, 768:1024]
                p = cell_pool.tile([128, 256], F16, tag="ph")
                nc.vector.tensor_mul(p[0:P], i_, gg)
                pr = cell_pool.tile([128, 256], F16, tag="prh")
                nc.vector.scalar_tensor_tensor(pr[0:P], p[0:P], 2.0, i_,
                                               ALU.mult, ALU.subtract)
                q = cell_pool.tile([128, 256], F16, tag="qh")
                nc.vector.tensor_mul(q[0:P], f_, cin_half)
                cn = cell_pool.tile([128, 256], F16, tag="cnh", bufs=3)
                nc.vector.tensor_add(cn[0:P], q[0:P], pr[0:P])
                tc_ = cell_pool.tile([128, 256], F16, tag="tch")
                nc.scalar.activation(tc_[0:P], cn[0:P], AF.Tanh)
                hn = cell_pool.tile([128, 256], F16, tag="hnh", bufs=3)
                nc.vector.tensor_mul(hn[0:P], o_, tc_[0:P])
                nc.gpsimd.dma_start(out_ap, hn[0:P])
                return (hn, cn, P)

            def feed(parent, u, ci):
                """Write u's crit states into parent stationary storage."""
                hn, cn, P = u
                half = P // 2
                base = ci * 64
                pid = eyp_s[0:P, 0:P]
                t0 = tr_pool.tile([128, 256], F16, tag="t")
                nc.tensor.transpose(t0[0:128, 0:P], hn[0:P, 0:128], pid)
                t1 = tr_pool.tile([128, 256], F16, tag="t")
                nc.tensor.transpose(t1[0:128, 0:P], hn[0:P, 128:256], pid)
                nc.vector.tensor_copy(parent.sA0[:, base:base + half],
                                      t0[:, 0:half])
                nc.vector.tensor_copy(parent.sB0[:, base:base + half],
                                      t0[:, half:P])
                nc.vector.tensor_copy(parent.sA1[:, base:base + half],
                                      t1[:, 0:half])
                nc.vector.tensor_copy(parent.sB1[:, base:base + half],
                                      t1[:, half:P])
                dr = base % 128
                cb = (ci // 2) * 512
                nc.sync.dma_start(parent.cin[dr:dr + half, cb:cb + 256],
                                  cn[0:P:2, 0:256])
                nc.sync.dma_start(parent.cin[dr:dr + half, cb + 256:cb + 512],
                                  cn[1:P:2, 0:256])

            # ---- emission schedule: L12 units interleave into L13's
            # ACT-saturated stretch (PE is idle there); feeds pace them ----
            u13 = [None] * 8
            u12 = [None] * 4
            u13[0] = unit_fused(13, 0, 0)
            u13[1] = unit_fused(13, 1, 0)
            u13[2] = unit_fused(13, 2, 0)
            u13[3] = unit_fused(13, 3, 0)
            feed(stor[12], u13[0], 0)
            feed(stor[12], u13[1], 1)
            u12[0] = unit_fused(12, 0, 1024)
            u13[4] = unit_fused(13, 4, 0)
            u13[5] = unit_fused(13, 5, 0)
            feed(stor[12], u13[2], 2)
            feed(stor[12], u13[3], 3)
            u12[1] = unit_fused(12, 1, 1024)
            u13[6] = unit_fused(13, 6, 0)
            u13[7] = unit_fused(13, 7, 0)
            feed(stor[12], u13[4], 4)
            feed(stor[12], u13[5], 5)
            u12[2] = unit_fused(12, 2, 1024)
            feed(stor[12], u13[6], 6)
            feed(stor[12], u13[7], 7)
            u12[3] = unit_fused(12, 3, 1024)
            feed(stor[11], u12[0], 0)
            feed(stor[11], u12[1], 1)
            u11_0 = unit_fused(11, 0, 1536)
            feed(stor[11], u12[2], 2)
            feed(stor[11], u12[3], 3)
            u11_1 = unit_fused(11, 1, 1536)
            feed(stor[10], u11_0, 0)
            feed(stor[10], u11_1, 1)

            # level 10: critical half (host boundary) then deferred half
            lhs10 = [stor[10].sA0, stor[10].sA1, stor[10].sB0, stor[10].sB1,
                     ohs_s[:, OHS_OFF[10]:OHS_OFF[10] + 128]]
            u10 = unit_half(lhs10, stor[10].cin[0:128, 0:256], 128,
                            0, out_d[1792:1920, 0:256])
            unit_half(lhs10, stor[10].cin[0:128, 256:512], 128,
                      1, out_d[1792:1920, 256:512])
            # level-10 c_crit rows for the host's top-of-tree chain
            nc.gpsimd.dma_start(out_d[1920:2048, 0:256], u10[1][0:128, 0:256])

    nc.compile()
    return [k for k in din]


def _get_built():
    global _BUILT
    if _BUILT is None:
        nc = bacc.Bacc("TRN2", target_bir_lowering=False, debug=False,
                       num_devices=N_CORES)
        names = _build_program(nc)
        _BUILT = (nc, names)
    return _BUILT


def _make_eyp():
    eyp = np.zeros((128, EYP_W), np.float16)
    for P, off in EYP_OFF.items():
        half = P // 2
        for bcol in range(P):
            a = 2 * bcol if bcol < half else 2 * (bcol - half) + 1
            eyp[a, off + bcol] = 1.0
    return eyp


def kernel(types, a_idx, b_idx, emb, W_ih, W_hh, b_ih, b_hh):
    types = np.asarray(types, np.int32)
    emb = np.asarray(emb, np.float32)
    W_ih = np.asarray(W_ih, np.float32)
    W_hh = np.asarray(W_hh, np.float32)
    b = np.asarray(b_ih, np.float32) + np.asarray(b_hh, np.float32)

    # ---- host weight reparameterization (O(V), no O(N) arithmetic) ----
    XT = (W_ih @ emb.T + b[:, None]).astype(np.float32)          # [2048, 32]
    c_leaf = _sigmoid(XT[0:512]) * np.tanh(XT[1024:1536])        # [512, 32]
    h_leaf = _sigmoid(XT[1536:2048]) * np.tanh(c_leaf)           # [512, 32]
    M_A = W_hh[:, 0:256] @ h_leaf[0:256]                         # [2048, 32]
    M_B = W_hh[:, 256:512] @ h_leaf[0:256]

    def dev_layout(mat_t):
        """[K, 2048] original gate cols -> [i|f|o|g], g cols doubled, f16."""
        m = np.ascontiguousarray(mat_t[:, PERM3], np.float32)
        m[:, 1536:2048] *= 2.0
        return m.astype(np.float16)

    w13 = dev_layout(np.vstack([M_A.T, M_B.T, XT.T]))            # [96, 2048]
    W_augT = dev_layout(np.vstack([W_hh.T, XT.T]))               # [544, 2048]
    wkh = [np.ascontiguousarray(W_augT[i * 128:(i + 1) * 128])
           for i in range(4)]
    woh = np.ascontiguousarray(W_augT[512:544])
    cl256 = np.ascontiguousarray(c_leaf[0:256].T, np.float16)    # [32, 256]
    eyp = _make_eyp()

    in_maps = []
    for j in range(N_CORES):
        base13 = (1 << 13) - 1 + j * 1024
        n = np.arange(base13, base13 + 1024)
        oh3 = np.zeros((96, 1024), np.float16)
        m = np.arange(1024)
        oh3[types[2 * n + 1], m] = 1.0
        oh3[32 + types[2 * n + 2], m] = 1.0
        oh3[64 + types[n], m] = 1.0
        cin13 = np.concatenate(
            [cl256[types[2 * n + 1]], cl256[types[2 * n + 2]]],
            axis=1).astype(np.float16)
        ohs = np.zeros((32, OHS_W), np.float16)
        for L in range(12, 9, -1):
            mm = 1 << (L - 3)
            basel = (1 << L) - 1 + j * mm
            off = OHS_OFF[L]
            ohs[types[basel:basel + mm], off + np.arange(mm)] = 1.0
        in_maps.append({
            "wk0": wkh[0], "wk1": wkh[1], "wk2": wkh[2], "wk3": wkh[3],
            "woh": woh, "w13": w13, "cin13": cin13,
            "oh3": oh3, "ohs": ohs, "eyp": eyp,
        })

    nc, _ = _get_built()
    res = run_bass_kernel_spmd(nc, in_maps, core_ids=list(range(N_CORES)))
    global LAST_RESULTS
    LAST_RESULTS = res

    out = np.empty((N, H2), np.float32)
    for j in range(N_CORES):
        r = res.results[j]["out"].astype(np.float32)
        for (L, mm, off) in DEV_PLAN:
            basel = (1 << L) - 1 + j * mm
            out[basel:basel + mm] = r[off:off + mm]
    out[LEAF0:] = h_leaf.T[types[LEAF0:]]

    # top of tree (levels 9..0, 1023 nodes) on host, mirroring the reference
    NB = (1 << 11) - 1  # nodes 0..2046 (level-10 boundary included)
    Hs = np.zeros((NB, H2), np.float32)
    Cc = np.zeros((NB, H), np.float32)  # c_crit only
    for j in range(N_CORES):
        r = res.results[j]["out"].astype(np.float32)
        b10 = (1 << 10) - 1 + 128 * j
        Hs[b10:b10 + 128] = r[1792:1920]
        Cc[b10:b10 + 128] = r[1920:2048, 0:256]
    for L in range(9, -1, -1):
        n = np.arange((1 << L) - 1, (1 << (L + 1)) - 1)
        a, bb = 2 * n + 1, 2 * n + 2
        h_in = np.concatenate([Hs[a, :H], Hs[bb, :H]], axis=1)
        c_in = np.concatenate([Cc[a], Cc[bb]], axis=1)
        gates = XT[:, types[n]].T + h_in @ W_hh.T
        ig, fg, gg, og = np.split(gates, 4, axis=1)
        c_new = _sigmoid(fg) * c_in + _sigmoid(ig) * np.tanh(gg)
        h_new = _sigmoid(og) * np.tanh(c_new)
        Hs[n] = h_new
        Cc[n] = c_new[:, 0:256]
        out[n] = h_new
    return out
